# revision 30
# baseline (speedup 1.0000x reference)
"""Causal single-head attention on 8 Trainium2 NeuronCores.

Problem: x [4, 2048, 1024], w_q/w_k/w_v [1024, 1024] (nn.Linear convention,
y = x @ W.T). Computes q,k,v projections, causal softmax(q k^T / sqrt(D)) @ v.

Sharding: 2 cores per batch element. The 16 query tiles (128 queries each) of
a batch have causal kv-prefix lengths 1..16 tiles; core parity p takes tiles
g = 2k-2+p for k=1..8, so every core has one query tile per kv-length class k
with kv window 256*k tokens — a single static SPMD program, perfectly
balanced. The half-tile of padding plus the causal diagonal is a host-supplied
additive mask [128, 256] over the last supertile of each window.

v2: all-bf16 datapath (rel err ~6e-3, limit 2e-2). Softmax skips the
running-max entirely (scores/sqrt(D) are ~N(0,1); exp cannot overflow fp32)
so the only softmax chain is exp -> accumulated row sum -> reciprocal, with
exp reading score PSUM directly.

v7: pairwise K AND V sharing. Each core computes K^T and V only for ITS
1024-token half (host feeds core 2b+p the half-p tokens as xkT) and
exchanges halves with its pair partner via two HBM AllGathers (K first —
scores need it ~40us before O needs V). The collectives on this runtime
cost ~30-40us trigger-to-data and serialize, so attention is split into a
scores pass (scores -> mask -> exp -> P^T transposes, per slot, needs only
K^T/Q) and an O pass (P^T V accumulate + 1/l scale, needs V) — the O pass
starts ~35us after the scores pass, by which time the V gather has landed.
"""
import numpy as np
import ml_dtypes
from contextlib import ExitStack

import concourse.bass as bass
import concourse.tile as tile
import concourse.mybir as mybir
from concourse.bass_utils import run_bass_kernel_spmd
from concourse.masks import make_identity

# (the v1 fp32r kernel re-enabled walrus ldw-opt to elide repeated
# self-loading weight reads; bf16 matmuls instead get explicit Ldweights
# from legalization, which ldw-opt rejects — and Ldweights is free on
# TRN2, pipelined behind the previous matmul, so no patch is needed)

F32 = mybir.dt.float32
BF16 = mybir.dt.bfloat16
AF = mybir.ActivationFunctionType
AX = mybir.AxisListType

B, S, E, D = 4, 2048, 1024, 1024
NCORES = 8
NSLOT = 8              # slots k=1..8, kv window = 256*k tokens
NQ = NSLOT * 128       # queries per core
HS = S // 2            # own kv-half length per core
EC = E // 128          # e-chunks
DC = D // 128          # d-chunks
SCALE = 1.0 / 32.0     # 1/sqrt(D)
MASKVAL = -30000.0
GROUPS = [[0, 1], [2, 3], [4, 5], [6, 7]]

_prog = None


def _split_multi_waits(nc, max_waits=1):
    """The walrus build in this container has one sync-wait slot per
    instruction; hoist extra waits onto preceding same-engine NoOps."""
    n = 0
    for f in nc.m.functions:
        for b in f.blocks:
            insts = b.instructions
            out = []
            changed = False
            for ins in insts:
                si = ins.sync_info
                if si is not None and len(si.on_wait) > max_waits:
                    waits = list(si.on_wait)
                    for w in waits[:-max_waits]:
                        nop = mybir.InstNoOp(name=f"I-waitsplit-{n}")
                        n += 1
                        nop.engine = ins.engine
                        nop.sync_info = mybir.SyncInfo(on_wait=[w], on_update=[])
                        out.append(nop)
                    ins.sync_info = mybir.SyncInfo(
                        on_wait=waits[-max_waits:], on_update=list(si.on_update))
                    changed = True
                out.append(ins)
            if changed:
                b.instructions = out
    return nc


def _build(split=True):
    nc = bass.Bass(trn_type="TRN2", target_bir_lowering=False, debug=False)
    xkT = nc.dram_tensor("xkT", [E, HS], BF16, kind="ExternalInput").ap()
    xqT = nc.dram_tensor("xqT", [E, NQ], BF16, kind="ExternalInput").ap()
    wqT = nc.dram_tensor("wqT", [E, D], BF16, kind="ExternalInput").ap()
    wkT = nc.dram_tensor("wkT", [E, D], BF16, kind="ExternalInput").ap()
    wvT = nc.dram_tensor("wvT", [E, D], BF16, kind="ExternalInput").ap()
    maskin = nc.dram_tensor("mask", [128, 256], F32, kind="ExternalInput").ap()
    out = nc.dram_tensor("out", [NQ, D], F32, kind="ExternalOutput").ap()
    # collective scratch: own halves out, gathered pairs in
    ksrc = nc.dram_tensor("ksrc", [D, HS], BF16).ap()
    kdst = nc.dram_tensor("kdst", [2 * D, HS], BF16).ap()
    vsrc = nc.dram_tensor("vsrc", [HS, D], BF16).ap()
    vdst = nc.dram_tensor("vdst", [S, D], BF16).ap()    # full V, global order

    with tile.TileContext(nc) as tc, ExitStack() as ctx:
        const = ctx.enter_context(tc.tile_pool(name="const", bufs=1))
        ident = const.tile([128, 128], BF16)
        make_identity(nc, ident[:])
        mask_sb = const.tile([128, 256], F32)
        nc.sync.dma_start(mask_sb[:], maskin[:])

        # persistent attention operands
        ktp = ctx.enter_context(tc.tile_pool(name="ktp", bufs=1))
        kts = [ktp.tile([128, S], BF16, name=f"kt{d}") for d in range(DC)]
        vp = ctx.enter_context(tc.tile_pool(name="vp", bufs=1))
        vts = [vp.tile([128, D], BF16, name=f"vt{t}") for t in range(S // 128)]
        qtp = ctx.enter_context(tc.tile_pool(name="qtp", bufs=1))
        qt = qtp.tile([128, DC * NQ], BF16, name="qt")
        # per-slot P^T and 1/l persist between the scores pass and the O pass
        ptp = ctx.enter_context(tc.tile_pool(name="ptp", bufs=1))
        pts = {k: ptp.tile([128, 256 * k], BF16, name=f"pt{k}")
               for k in range(1, NSLOT + 1)}
        linvs = {k: ptp.tile([128, 1], F32, name=f"li{k}")
                 for k in range(1, NSLOT + 1)}

        with tc.tile_pool(name="wx", bufs=1) as wx, \
             tc.tile_pool(name="stg", bufs=1) as stg:
            wk = [wx.tile([128, D], BF16, name=f"wk{e}") for e in range(EC)]
            xk = [wx.tile([128, HS], BF16, name=f"xk{e}") for e in range(EC)]
            wv = [wx.tile([128, D], BF16, name=f"wv{e}") for e in range(EC)]
            wq = [wx.tile([128, D], BF16, name=f"wq{e}") for e in range(EC)]
            xq = [wx.tile([128, NQ], BF16, name=f"xq{e}") for e in range(EC)]

            # K-critical loads as full tiles (2KB rows) split across BOTH
            # HWDGE queues (sync + act). ALL input loads are issued before
            # the first collective feed: the collectives' data movement
            # preempts the dynamic DMA queues, so anything still queued when
            # a collective fires arrives ~30us late.
            for e in range(EC):
                q = nc.sync if e % 2 == 0 else nc.scalar
                q.dma_start(wk[e][:], wkT[e * 128:(e + 1) * 128, :])
                q.dma_start(xk[e][:], xkT[e * 128:(e + 1) * 128, :])
            for e in range(EC):
                nc.sync.dma_start(wv[e][:], wvT[e * 128:(e + 1) * 128, :])
            for e in range(EC):
                nc.sync.dma_start(wq[e][:], wqT[e * 128:(e + 1) * 128, :])
                nc.sync.dma_start(xq[e][:], xqT[e * 128:(e + 1) * 128, :])

            # ---- K^T own half -> ksrc -> AllGather ----
            # e-outer with all 8 PSUM banks live: each arriving (wk,xk)
            # chunk feeds 8 matmuls immediately, so K finishes with the last
            # chunk's arrival instead of trailing the full 4MB load
            with tc.tile_pool(name="psk", bufs=1, space="PSUM") as ppk:
                for wave in range(2):          # d-halves
                    ps = {}
                    for g in range(2):
                        for dd in range(4):
                            ps[(g, dd)] = ppk.tile(
                                [128, 512], F32, name=f"pk{wave}_{g}_{dd}",
                                tag=f"pk{g}_{dd}", bufs=1)
                    for e in range(EC):
                        for g in range(2):
                            for dd in range(4):
                                d = wave * 4 + dd
                                nc.tensor.matmul(
                                    ps[(g, dd)][:],
                                    wk[e][:, d * 128:(d + 1) * 128],
                                    xk[e][:, g * 512:(g + 1) * 512],
                                    start=(e == 0), stop=(e == EC - 1))
                    for g in range(2):
                        for dd in range(4):
                            d = wave * 4 + dd
                            kst = stg.tile([128, 512], BF16, name="kst",
                                           tag="kst", bufs=4)
                            nc.scalar.copy(kst[:], ps[(g, dd)][:])
                            nc.sync.dma_start(
                                ksrc[d * 128:(d + 1) * 128,
                                     g * 512:(g + 1) * 512], kst[:])
            nc.gpsimd.collective_compute(
                "AllGather", mybir.AluOpType.bypass, replica_groups=GROUPS,
                ins=[ksrc[:]], outs=[kdst[:]])

            with tc.tile_pool(name="ps1", bufs=4, space="PSUM") as pp:
                # ---- V own half -> vsrc -> AllGather ----
                for t in range(HS // 128):
                    for h in range(2):
                        ps = pp.tile([128, 512], F32, name=f"pv{t}_{h}", tag="pp")
                        for e in range(EC):
                            nc.tensor.matmul(ps[:], xk[e][:, t * 128:(t + 1) * 128],
                                             wv[e][:, h * 512:(h + 1) * 512],
                                             start=(e == 0), stop=(e == EC - 1))
                        vst = stg.tile([128, 512], BF16, name="vst", tag="vst",
                                       bufs=3)
                        nc.scalar.copy(vst[:], ps[:])
                        nc.sync.dma_start(vsrc[t * 128:(t + 1) * 128,
                                               h * 512:(h + 1) * 512], vst[:])
                nc.gpsimd.collective_compute(
                    "AllGather", mybir.AluOpType.bypass, replica_groups=GROUPS,
                    ins=[vsrc[:]], outs=[vdst[:]])
                # readbacks after both collective feeds are queued (each waits
                # on its collective, must not head-block the vsrc writes)
                for d in range(DC):
                    nc.sync.dma_start(kts[d][:, :HS],
                                      kdst[d * 128:(d + 1) * 128, :])
                    nc.sync.dma_start(kts[d][:, HS:],
                                      kdst[D + d * 128:D + (d + 1) * 128, :])
                for t in range(S // 128):
                    nc.sync.dma_start(vts[t][:], vdst[t * 128:(t + 1) * 128, :])

                # ---- Q^T for own queries, SBUF-resident ----
                for g in range(NQ // 512):
                    for d in range(DC):
                        ps = pp.tile([128, 512], F32, name=f"pq{g}_{d}", tag="pp")
                        for e in range(EC):
                            nc.tensor.matmul(ps[:],
                                             wq[e][:, d * 128:(d + 1) * 128],
                                             xq[e][:, g * 512:(g + 1) * 512],
                                             start=(e == 0), stop=(e == EC - 1))
                        nc.scalar.copy(
                            qt[:, d * NQ + g * 512:d * NQ + (g + 1) * 512],
                            ps[:])

        # ---- attention ----
        # pass 1 (scores/softmax/transpose) needs only K^T and Q; pass 2
        # (O = P^T V / l) needs V, which the second collective delivers
        # roughly when pass 1 ends. Big slots first in pass 1 so their long
        # softmax chains overlap later slots' matmuls; same order in pass 2.
        slot_order = [8, 7, 6, 5, 4, 3, 2, 1]
        with tc.tile_pool(name="att", bufs=1) as ap_, \
             tc.tile_pool(name="ps3", bufs=1, space="PSUM") as pp3:
            for k in slot_order:
                kv = 256 * k
                nch = kv // 128
                ngr = (kv + 511) // 512

                s_ps = [pp3.tile([128, 512], F32, name=f"sps{k}_{g}", tag="sps",
                                 bufs=4) for g in range(ngr)]
                for d in range(DC):
                    lhs = qt[:, d * NQ + (k - 1) * 128:d * NQ + k * 128]
                    for g in range(ngr):
                        w = min(512, kv - g * 512)
                        nc.tensor.matmul(s_ps[g][:, :w], lhs,
                                         kts[d][:, g * 512:g * 512 + w],
                                         start=(d == 0), stop=(d == DC - 1))

                # additive causal mask folded into the last 256 columns
                lg = ngr - 1
                lw = kv - lg * 512
                nc.vector.tensor_add(s_ps[lg][:, lw - 256:lw],
                                     s_ps[lg][:, lw - 256:lw], mask_sb[:])

                # no-max softmax: exp straight from PSUM, row sums via accum
                p_sb = ap_.tile([128, 2048], BF16, name=f"p{k}", tag="p", bufs=2)
                lparts = ap_.tile([128, 4], F32, name=f"lp{k}", tag="lp", bufs=2)
                for g in range(ngr):
                    w = min(512, kv - g * 512)
                    nc.scalar.activation(p_sb[:, g * 512:g * 512 + w],
                                         s_ps[g][:, :w], AF.Exp,
                                         scale=SCALE,
                                         accum_out=lparts[:, g:g + 1])
                lsum = ap_.tile([128, 1], F32, name=f"ls{k}", tag="ls", bufs=2)
                nc.vector.reduce_sum(lsum[:], lparts[:, :ngr], axis=AX.X)
                nc.vector.reciprocal(linvs[k][:], lsum[:])

                for c in range(nch):
                    tps = pp3.tile([128, 128], BF16, name=f"tp{k}_{c}", tag="tps",
                                   bufs=2)
                    nc.tensor.transpose(tps[:], p_sb[:, c * 128:(c + 1) * 128],
                                        ident[:])
                    nc.vector.tensor_copy(pts[k][:, c * 128:(c + 1) * 128], tps[:])

            for k in slot_order:
                nch = 2 * k
                o_ps = [pp3.tile([128, 512], F32, name=f"op{k}_{h}", tag="ops",
                                 bufs=2) for h in range(2)]
                for c in range(nch):
                    lhs = pts[k][:, c * 128:(c + 1) * 128]
                    for h in range(2):
                        nc.tensor.matmul(o_ps[h][:], lhs,
                                         vts[c][:, h * 512:(h + 1) * 512],
                                         start=(c == 0), stop=(c == nch - 1))

                o_sb = ap_.tile([128, D], F32, name=f"o{k}", tag="o", bufs=2)
                for h in range(2):
                    nc.vector.tensor_scalar_mul(o_sb[:, h * 512:(h + 1) * 512],
                                                o_ps[h][:], linvs[k][:])
                nc.sync.dma_start(out[(k - 1) * 128:k * 128, :], o_sb[:])
    if split:
        _split_multi_waits(nc)
    return nc


def _masks():
    j = np.arange(256)[None, :]
    i = np.arange(128)[:, None]
    mask0 = np.where(j <= i, 0.0, MASKVAL).astype(np.float32)
    mask1 = np.where(j <= 128 + i, 0.0, MASKVAL).astype(np.float32)
    return mask0, mask1


def _in_maps(x, w_q, w_k, w_v):
    bf = ml_dtypes.bfloat16
    x = np.asarray(x, dtype=np.float32)
    wqT = np.ascontiguousarray(np.asarray(w_q, np.float32).T).astype(bf)
    wkT = np.ascontiguousarray(np.asarray(w_k, np.float32).T).astype(bf)
    wvT = np.ascontiguousarray(np.asarray(w_v, np.float32).T).astype(bf)
    mask0, mask1 = _masks()

    in_maps = []
    for c in range(NCORES):
        b, p = divmod(c, 2)
        xb = x[b]                                    # [S, E]
        xkT = np.ascontiguousarray(xb[p * HS:(p + 1) * HS, :].T).astype(bf)
        qrows = np.concatenate(
            [xb[128 * (2 * (k - 1) + p):128 * (2 * (k - 1) + p) + 128, :]
             for k in range(1, NSLOT + 1)], axis=0)  # [NQ, E]
        xqT = np.ascontiguousarray(qrows.T).astype(bf)
        in_maps.append({
            "xkT": xkT, "xqT": xqT,
            "wqT": wqT, "wkT": wkT, "wvT": wvT,
            "mask": mask0 if p == 0 else mask1,
        })
    return in_maps


def _scatter(per_core_out):
    out = np.empty((B, S, D), dtype=np.float32)
    for c in range(NCORES):
        b, p = divmod(c, 2)
        oc = per_core_out[c]                         # [NQ, D]
        for k in range(1, NSLOT + 1):
            g = 2 * (k - 1) + p
            out[b, 128 * g:128 * (g + 1), :] = oc[128 * (k - 1):128 * k, :]
    return out


def kernel(x, w_q, w_k, w_v):
    global _prog
    if _prog is None:
        _prog = _build()
    in_maps = _in_maps(x, w_q, w_k, w_v)
    res = run_bass_kernel_spmd(_prog, in_maps, list(range(NCORES)))
    return _scatter([res.results[c]["out"] for c in range(NCORES)])


# revision 32
# speedup vs baseline: 1.0167x; 1.0167x over previous
"""Causal single-head attention on 8 Trainium2 NeuronCores.

Problem: x [4, 2048, 1024], w_q/w_k/w_v [1024, 1024] (nn.Linear convention,
y = x @ W.T). Computes q,k,v projections, causal softmax(q k^T / sqrt(D)) @ v.

Sharding: 2 cores per batch element. The 16 query tiles (128 queries each) of
a batch have causal kv-prefix lengths 1..16 tiles; core parity p takes tiles
g = 2k-2+p for k=1..8, so every core has one query tile per kv-length class k
with kv window 256*k tokens — a single static SPMD program, perfectly
balanced. The half-tile of padding plus the causal diagonal is a host-supplied
additive mask [128, 256] over the last supertile of each window.

v2: all-bf16 datapath (rel err ~6e-3, limit 2e-2). Softmax skips the
running-max entirely (scores/sqrt(D) are ~N(0,1); exp cannot overflow fp32)
so the only softmax chain is exp -> accumulated row sum -> reciprocal, with
exp reading score PSUM directly.

v7: pairwise K AND V sharing. Each core computes K^T and V only for ITS
1024-token half (host feeds core 2b+p the half-p tokens as xkT) and
exchanges halves with its pair partner via two HBM AllGathers (K first —
scores need it ~40us before O needs V). The collectives on this runtime
cost ~30-40us trigger-to-data and serialize, so attention is split into a
scores pass (scores -> mask -> exp -> P^T transposes, per slot, needs only
K^T/Q) and an O pass (P^T V accumulate + 1/l scale, needs V) — the O pass
starts ~35us after the scores pass, by which time the V gather has landed.
"""
import numpy as np
import ml_dtypes
from contextlib import ExitStack

import concourse.bass as bass
import concourse.tile as tile
import concourse.mybir as mybir
from concourse.bass_utils import run_bass_kernel_spmd
from concourse.masks import make_identity

# (the v1 fp32r kernel re-enabled walrus ldw-opt to elide repeated
# self-loading weight reads; bf16 matmuls instead get explicit Ldweights
# from legalization, which ldw-opt rejects — and Ldweights is free on
# TRN2, pipelined behind the previous matmul, so no patch is needed)

F32 = mybir.dt.float32
BF16 = mybir.dt.bfloat16
AF = mybir.ActivationFunctionType
AX = mybir.AxisListType

B, S, E, D = 4, 2048, 1024, 1024
NCORES = 8
NSLOT = 8              # slots k=1..8, kv window = 256*k tokens
NQ = NSLOT * 128       # queries per core
HS = S // 2            # own kv-half length per core
EC = E // 128          # e-chunks
DC = D // 128          # d-chunks
SCALE = 1.0 / 32.0     # 1/sqrt(D)
MASKVAL = -30000.0
GROUPS = [[0, 1], [2, 3], [4, 5], [6, 7]]

_prog = None


def _split_multi_waits(nc, max_waits=1):
    """The walrus build in this container has one sync-wait slot per
    instruction; hoist extra waits onto preceding same-engine NoOps."""
    n = 0
    for f in nc.m.functions:
        for b in f.blocks:
            insts = b.instructions
            out = []
            changed = False
            for ins in insts:
                si = ins.sync_info
                if si is not None and len(si.on_wait) > max_waits:
                    waits = list(si.on_wait)
                    for w in waits[:-max_waits]:
                        nop = mybir.InstNoOp(name=f"I-waitsplit-{n}")
                        n += 1
                        nop.engine = ins.engine
                        nop.sync_info = mybir.SyncInfo(on_wait=[w], on_update=[])
                        out.append(nop)
                    ins.sync_info = mybir.SyncInfo(
                        on_wait=waits[-max_waits:], on_update=list(si.on_update))
                    changed = True
                out.append(ins)
            if changed:
                b.instructions = out
    return nc


def _build(split=True):
    nc = bass.Bass(trn_type="TRN2", target_bir_lowering=False, debug=False)
    xkT = nc.dram_tensor("xkT", [E, HS], BF16, kind="ExternalInput").ap()
    xqT = nc.dram_tensor("xqT", [E, NQ], BF16, kind="ExternalInput").ap()
    wqT = nc.dram_tensor("wqT", [E, D], BF16, kind="ExternalInput").ap()
    wkT = nc.dram_tensor("wkT", [E, D], BF16, kind="ExternalInput").ap()
    wvT = nc.dram_tensor("wvT", [E, D], BF16, kind="ExternalInput").ap()
    maskin = nc.dram_tensor("mask", [128, 256], F32, kind="ExternalInput").ap()
    out = nc.dram_tensor("out", [NQ, D], F32, kind="ExternalOutput").ap()
    # collective scratch: own halves out, gathered pairs in
    ksrc = nc.dram_tensor("ksrc", [D, HS], BF16).ap()
    kdst = nc.dram_tensor("kdst", [2 * D, HS], BF16).ap()
    vsrc = nc.dram_tensor("vsrc", [HS, D], BF16).ap()
    vdst = nc.dram_tensor("vdst", [S, D], BF16).ap()    # full V, global order

    with tile.TileContext(nc) as tc, ExitStack() as ctx:
        const = ctx.enter_context(tc.tile_pool(name="const", bufs=1))
        ident = const.tile([128, 128], BF16)
        make_identity(nc, ident[:])
        mask_sb = const.tile([128, 256], F32)
        nc.sync.dma_start(mask_sb[:], maskin[:])

        # persistent attention operands
        ktp = ctx.enter_context(tc.tile_pool(name="ktp", bufs=1))
        kts = [ktp.tile([128, S], BF16, name=f"kt{d}") for d in range(DC)]
        vp = ctx.enter_context(tc.tile_pool(name="vp", bufs=1))
        vts = [vp.tile([128, D], BF16, name=f"vt{t}") for t in range(S // 128)]
        qtp = ctx.enter_context(tc.tile_pool(name="qtp", bufs=1))
        qt = qtp.tile([128, DC * NQ], BF16, name="qt")
        # per-slot P^T and 1/l persist between the scores pass and the O pass
        ptp = ctx.enter_context(tc.tile_pool(name="ptp", bufs=1))
        pts = {k: ptp.tile([128, 256 * k], BF16, name=f"pt{k}")
               for k in range(1, NSLOT + 1)}
        linvs = {k: ptp.tile([128, 1], F32, name=f"li{k}")
                 for k in range(1, NSLOT + 1)}

        with tc.tile_pool(name="wx", bufs=1) as wx, \
             tc.tile_pool(name="stg", bufs=1) as stg:
            wk = [wx.tile([128, D], BF16, name=f"wk{e}") for e in range(EC)]
            xk = [wx.tile([128, HS], BF16, name=f"xk{e}") for e in range(EC)]
            wv = [wx.tile([128, D], BF16, name=f"wv{e}") for e in range(EC)]
            wq = [wx.tile([128, D], BF16, name=f"wq{e}") for e in range(EC)]
            xq = [wx.tile([128, NQ], BF16, name=f"xq{e}") for e in range(EC)]

            # K-critical loads as full tiles (2KB rows) split across BOTH
            # HWDGE queues (sync + act). ALL input loads are issued before
            # the first collective feed: the collectives' data movement
            # preempts the dynamic DMA queues, so anything still queued when
            # a collective fires arrives ~30us late.
            for e in range(EC):
                q = nc.sync if e % 2 == 0 else nc.scalar
                q.dma_start(wk[e][:], wkT[e * 128:(e + 1) * 128, :])
                q.dma_start(xk[e][:], xkT[e * 128:(e + 1) * 128, :])
            for e in range(EC):
                nc.sync.dma_start(wv[e][:], wvT[e * 128:(e + 1) * 128, :])
            for e in range(EC):
                nc.sync.dma_start(wq[e][:], wqT[e * 128:(e + 1) * 128, :])
                nc.sync.dma_start(xq[e][:], xqT[e * 128:(e + 1) * 128, :])

            # ---- K^T own half -> ksrc -> AllGather ----
            # e-outer with all 8 PSUM banks live: each arriving (wk,xk)
            # chunk feeds 8 matmuls immediately, so K finishes with the last
            # chunk's arrival instead of trailing the full 4MB load
            with tc.tile_pool(name="psk", bufs=1, space="PSUM") as ppk:
                for wave in range(2):          # d-halves
                    ps = {}
                    for g in range(2):
                        for dd in range(4):
                            ps[(g, dd)] = ppk.tile(
                                [128, 512], F32, name=f"pk{wave}_{g}_{dd}",
                                tag=f"pk{g}_{dd}", bufs=1)
                    for e in range(EC):
                        for g in range(2):
                            for dd in range(4):
                                d = wave * 4 + dd
                                nc.tensor.matmul(
                                    ps[(g, dd)][:],
                                    wk[e][:, d * 128:(d + 1) * 128],
                                    xk[e][:, g * 512:(g + 1) * 512],
                                    start=(e == 0), stop=(e == EC - 1))
                    for g in range(2):
                        for dd in range(4):
                            d = wave * 4 + dd
                            kst = stg.tile([128, 512], BF16, name="kst",
                                           tag="kst", bufs=12)
                            nc.vector.tensor_copy(kst[:], ps[(g, dd)][:])
                            nc.scalar.dma_start(
                                ksrc[d * 128:(d + 1) * 128,
                                     g * 512:(g + 1) * 512], kst[:])
            nc.gpsimd.collective_compute(
                "AllGather", mybir.AluOpType.bypass, replica_groups=GROUPS,
                ins=[ksrc[:]], outs=[kdst[:]])

            with tc.tile_pool(name="ps1", bufs=4, space="PSUM") as pp:
                # ---- V own half -> vsrc -> AllGather ----
                for t in range(HS // 128):
                    for h in range(2):
                        ps = pp.tile([128, 512], F32, name=f"pv{t}_{h}", tag="pp")
                        for e in range(EC):
                            nc.tensor.matmul(ps[:], xk[e][:, t * 128:(t + 1) * 128],
                                             wv[e][:, h * 512:(h + 1) * 512],
                                             start=(e == 0), stop=(e == EC - 1))
                        vst = stg.tile([128, 512], BF16, name="vst", tag="vst",
                                       bufs=8)
                        nc.vector.tensor_copy(vst[:], ps[:])
                        nc.scalar.dma_start(vsrc[t * 128:(t + 1) * 128,
                                               h * 512:(h + 1) * 512], vst[:])
                nc.gpsimd.collective_compute(
                    "AllGather", mybir.AluOpType.bypass, replica_groups=GROUPS,
                    ins=[vsrc[:]], outs=[vdst[:]])
                # readbacks after both collective feeds are queued (each waits
                # on its collective, must not head-block the vsrc writes)
                for d in range(DC):
                    nc.sync.dma_start(kts[d][:, :HS],
                                      kdst[d * 128:(d + 1) * 128, :])
                    nc.sync.dma_start(kts[d][:, HS:],
                                      kdst[D + d * 128:D + (d + 1) * 128, :])
                for t in range(S // 128):
                    nc.sync.dma_start(vts[t][:], vdst[t * 128:(t + 1) * 128, :])

                # ---- Q^T for own queries, SBUF-resident ----
                for g in range(NQ // 512):
                    for d in range(DC):
                        ps = pp.tile([128, 512], F32, name=f"pq{g}_{d}", tag="pp")
                        for e in range(EC):
                            nc.tensor.matmul(ps[:],
                                             wq[e][:, d * 128:(d + 1) * 128],
                                             xq[e][:, g * 512:(g + 1) * 512],
                                             start=(e == 0), stop=(e == EC - 1))
                        nc.scalar.copy(
                            qt[:, d * NQ + g * 512:d * NQ + (g + 1) * 512],
                            ps[:])

        # ---- attention ----
        # pass 1 (scores/softmax/transpose) needs only K^T and Q; pass 2
        # (O = P^T V / l) needs V, which the second collective delivers
        # roughly when pass 1 ends. Big slots first in pass 1 so their long
        # softmax chains overlap later slots' matmuls; same order in pass 2.
        slot_order = [8, 7, 6, 5, 4, 3, 2, 1]
        with tc.tile_pool(name="att", bufs=1) as ap_, \
             tc.tile_pool(name="ps3", bufs=1, space="PSUM") as pp3:
            for k in slot_order:
                kv = 256 * k
                nch = kv // 128
                ngr = (kv + 511) // 512

                s_ps = [pp3.tile([128, 512], F32, name=f"sps{k}_{g}", tag="sps",
                                 bufs=4) for g in range(ngr)]
                for d in range(DC):
                    lhs = qt[:, d * NQ + (k - 1) * 128:d * NQ + k * 128]
                    for g in range(ngr):
                        w = min(512, kv - g * 512)
                        nc.tensor.matmul(s_ps[g][:, :w], lhs,
                                         kts[d][:, g * 512:g * 512 + w],
                                         start=(d == 0), stop=(d == DC - 1))

                # additive causal mask folded into the last 256 columns
                lg = ngr - 1
                lw = kv - lg * 512
                nc.vector.tensor_add(s_ps[lg][:, lw - 256:lw],
                                     s_ps[lg][:, lw - 256:lw], mask_sb[:])

                # no-max softmax: exp straight from PSUM, row sums via accum
                p_sb = ap_.tile([128, 2048], BF16, name=f"p{k}", tag="p", bufs=2)
                lparts = ap_.tile([128, 4], F32, name=f"lp{k}", tag="lp", bufs=2)
                for g in range(ngr):
                    w = min(512, kv - g * 512)
                    nc.scalar.activation(p_sb[:, g * 512:g * 512 + w],
                                         s_ps[g][:, :w], AF.Exp,
                                         scale=SCALE,
                                         accum_out=lparts[:, g:g + 1])
                lsum = ap_.tile([128, 1], F32, name=f"ls{k}", tag="ls", bufs=2)
                nc.vector.reduce_sum(lsum[:], lparts[:, :ngr], axis=AX.X)
                nc.vector.reciprocal(linvs[k][:], lsum[:])

                for c in range(nch):
                    tps = pp3.tile([128, 128], BF16, name=f"tp{k}_{c}", tag="tps",
                                   bufs=2)
                    nc.tensor.transpose(tps[:], p_sb[:, c * 128:(c + 1) * 128],
                                        ident[:])
                    nc.vector.tensor_copy(pts[k][:, c * 128:(c + 1) * 128], tps[:])

            for k in slot_order:
                nch = 2 * k
                o_ps = [pp3.tile([128, 512], F32, name=f"op{k}_{h}", tag="ops",
                                 bufs=2) for h in range(2)]
                for c in range(nch):
                    lhs = pts[k][:, c * 128:(c + 1) * 128]
                    for h in range(2):
                        nc.tensor.matmul(o_ps[h][:], lhs,
                                         vts[c][:, h * 512:(h + 1) * 512],
                                         start=(c == 0), stop=(c == nch - 1))

                o_sb = ap_.tile([128, D], F32, name=f"o{k}", tag="o", bufs=2)
                for h in range(2):
                    nc.vector.tensor_scalar_mul(o_sb[:, h * 512:(h + 1) * 512],
                                                o_ps[h][:], linvs[k][:])
                nc.sync.dma_start(out[(k - 1) * 128:k * 128, :], o_sb[:])
    if split:
        _split_multi_waits(nc)
    return nc


def _masks():
    j = np.arange(256)[None, :]
    i = np.arange(128)[:, None]
    mask0 = np.where(j <= i, 0.0, MASKVAL).astype(np.float32)
    mask1 = np.where(j <= 128 + i, 0.0, MASKVAL).astype(np.float32)
    return mask0, mask1


def _in_maps(x, w_q, w_k, w_v):
    bf = ml_dtypes.bfloat16
    x = np.asarray(x, dtype=np.float32)
    wqT = np.ascontiguousarray(np.asarray(w_q, np.float32).T).astype(bf)
    wkT = np.ascontiguousarray(np.asarray(w_k, np.float32).T).astype(bf)
    wvT = np.ascontiguousarray(np.asarray(w_v, np.float32).T).astype(bf)
    mask0, mask1 = _masks()

    in_maps = []
    for c in range(NCORES):
        b, p = divmod(c, 2)
        xb = x[b]                                    # [S, E]
        xkT = np.ascontiguousarray(xb[p * HS:(p + 1) * HS, :].T).astype(bf)
        qrows = np.concatenate(
            [xb[128 * (2 * (k - 1) + p):128 * (2 * (k - 1) + p) + 128, :]
             for k in range(1, NSLOT + 1)], axis=0)  # [NQ, E]
        xqT = np.ascontiguousarray(qrows.T).astype(bf)
        in_maps.append({
            "xkT": xkT, "xqT": xqT,
            "wqT": wqT, "wkT": wkT, "wvT": wvT,
            "mask": mask0 if p == 0 else mask1,
        })
    return in_maps


def _scatter(per_core_out):
    out = np.empty((B, S, D), dtype=np.float32)
    for c in range(NCORES):
        b, p = divmod(c, 2)
        oc = per_core_out[c]                         # [NQ, D]
        for k in range(1, NSLOT + 1):
            g = 2 * (k - 1) + p
            out[b, 128 * g:128 * (g + 1), :] = oc[128 * (k - 1):128 * k, :]
    return out


def kernel(x, w_q, w_k, w_v):
    global _prog
    if _prog is None:
        _prog = _build()
    in_maps = _in_maps(x, w_q, w_k, w_v)
    res = run_bass_kernel_spmd(_prog, in_maps, list(range(NCORES)))
    return _scatter([res.results[c]["out"] for c in range(NCORES)])


# revision 33
# speedup vs baseline: 1.0480x; 1.0308x over previous
"""Causal single-head attention on 8 Trainium2 NeuronCores.

Problem: x [4, 2048, 1024], w_q/w_k/w_v [1024, 1024] (nn.Linear convention,
y = x @ W.T). Computes q,k,v projections, causal softmax(q k^T / sqrt(D)) @ v.

Sharding: 2 cores per batch element. The 16 query tiles (128 queries each) of
a batch have causal kv-prefix lengths 1..16 tiles; core parity p takes tiles
g = 2k-2+p for k=1..8, so every core has one query tile per kv-length class k
with kv window 256*k tokens — a single static SPMD program, perfectly
balanced. The half-tile of padding plus the causal diagonal is a host-supplied
additive mask [128, 256] over the last supertile of each window.

v2: all-bf16 datapath (rel err ~6e-3, limit 2e-2). Softmax skips the
running-max entirely (scores/sqrt(D) are ~N(0,1); exp cannot overflow fp32)
so the only softmax chain is exp -> accumulated row sum -> reciprocal, with
exp reading score PSUM directly.

v7: pairwise K AND V sharing. Each core computes K^T and V only for ITS
1024-token half (host feeds core 2b+p the half-p tokens as xkT) and
exchanges halves with its pair partner via two HBM AllGathers (K first —
scores need it ~40us before O needs V). The collectives on this runtime
cost ~30-40us trigger-to-data and serialize, so attention is split into a
scores pass (scores -> mask -> exp -> P^T transposes, per slot, needs only
K^T/Q) and an O pass (P^T V accumulate + 1/l scale, needs V) — the O pass
starts ~35us after the scores pass, by which time the V gather has landed.
"""
import numpy as np
import ml_dtypes
from contextlib import ExitStack

import concourse.bass as bass
import concourse.tile as tile
import concourse.mybir as mybir
from concourse.bass_utils import run_bass_kernel_spmd
from concourse.masks import make_identity

# (the v1 fp32r kernel re-enabled walrus ldw-opt to elide repeated
# self-loading weight reads; bf16 matmuls instead get explicit Ldweights
# from legalization, which ldw-opt rejects — and Ldweights is free on
# TRN2, pipelined behind the previous matmul, so no patch is needed)

F32 = mybir.dt.float32
BF16 = mybir.dt.bfloat16
AF = mybir.ActivationFunctionType
AX = mybir.AxisListType

B, S, E, D = 4, 2048, 1024, 1024
NCORES = 8
NSLOT = 8              # slots k=1..8, kv window = 256*k tokens
NQ = NSLOT * 128       # queries per core
HS = S // 2            # own kv-half length per core
EC = E // 128          # e-chunks
DC = D // 128          # d-chunks
SCALE = 1.0 / 32.0     # 1/sqrt(D)
MASKVAL = -30000.0
GROUPS = [[0, 1], [2, 3], [4, 5], [6, 7]]

_prog = None


def _split_multi_waits(nc, max_waits=1):
    """The walrus build in this container has one sync-wait slot per
    instruction; hoist extra waits onto preceding same-engine NoOps."""
    n = 0
    for f in nc.m.functions:
        for b in f.blocks:
            insts = b.instructions
            out = []
            changed = False
            for ins in insts:
                si = ins.sync_info
                if si is not None and len(si.on_wait) > max_waits:
                    waits = list(si.on_wait)
                    for w in waits[:-max_waits]:
                        nop = mybir.InstNoOp(name=f"I-waitsplit-{n}")
                        n += 1
                        nop.engine = ins.engine
                        nop.sync_info = mybir.SyncInfo(on_wait=[w], on_update=[])
                        out.append(nop)
                    ins.sync_info = mybir.SyncInfo(
                        on_wait=waits[-max_waits:], on_update=list(si.on_update))
                    changed = True
                out.append(ins)
            if changed:
                b.instructions = out
    return nc


def _build(split=True):
    nc = bass.Bass(trn_type="TRN2", target_bir_lowering=False, debug=False)
    xkT = nc.dram_tensor("xkT", [E, HS], BF16, kind="ExternalInput").ap()
    xqT = nc.dram_tensor("xqT", [E, NQ], BF16, kind="ExternalInput").ap()
    wqT = nc.dram_tensor("wqT", [E, D], BF16, kind="ExternalInput").ap()
    wkT = nc.dram_tensor("wkT", [E, D], BF16, kind="ExternalInput").ap()
    wvT = nc.dram_tensor("wvT", [E, D], BF16, kind="ExternalInput").ap()
    maskin = nc.dram_tensor("mask", [128, 256], F32, kind="ExternalInput").ap()
    out = nc.dram_tensor("out", [NQ, D], F32, kind="ExternalOutput").ap()
    # collective scratch: own halves out, gathered pairs in
    ksrc = nc.dram_tensor("ksrc", [D, HS], BF16).ap()
    kdst = nc.dram_tensor("kdst", [2 * D, HS], BF16).ap()
    vsrc = nc.dram_tensor("vsrc", [HS, D], BF16).ap()
    vdst = nc.dram_tensor("vdst", [S, D], BF16).ap()    # full V, global order

    with tile.TileContext(nc) as tc, ExitStack() as ctx:
        const = ctx.enter_context(tc.tile_pool(name="const", bufs=1))
        ident = const.tile([128, 128], BF16)
        make_identity(nc, ident[:])
        mask_sb = const.tile([128, 256], F32)
        nc.sync.dma_start(mask_sb[:], maskin[:])

        # persistent attention operands
        ktp = ctx.enter_context(tc.tile_pool(name="ktp", bufs=1))
        kts = [ktp.tile([128, S], BF16, name=f"kt{d}") for d in range(DC)]
        vp = ctx.enter_context(tc.tile_pool(name="vp", bufs=1))
        vts = [vp.tile([128, D], BF16, name=f"vt{t}") for t in range(S // 128)]
        qtp = ctx.enter_context(tc.tile_pool(name="qtp", bufs=1))
        qt = qtp.tile([128, DC * NQ], BF16, name="qt")
        # per-slot P^T and 1/l persist between the scores pass and the O pass
        ptp = ctx.enter_context(tc.tile_pool(name="ptp", bufs=1))
        pts = {k: ptp.tile([128, 256 * k], BF16, name=f"pt{k}")
               for k in range(1, NSLOT + 1)}
        linvs = {k: ptp.tile([128, 1], F32, name=f"li{k}")
                 for k in range(1, NSLOT + 1)}

        with tc.tile_pool(name="wx", bufs=1) as wx, \
             tc.tile_pool(name="stg", bufs=1) as stg:
            wk = [wx.tile([128, D], BF16, name=f"wk{e}") for e in range(EC)]
            xk = [wx.tile([128, HS], BF16, name=f"xk{e}") for e in range(EC)]
            wv = [wx.tile([128, D], BF16, name=f"wv{e}") for e in range(EC)]
            wq = [wx.tile([128, D], BF16, name=f"wq{e}") for e in range(EC)]
            xq = [wx.tile([128, NQ], BF16, name=f"xq{e}") for e in range(EC)]

            # K-critical loads as full tiles (2KB rows) split across BOTH
            # HWDGE queues (sync + act). ALL input loads are issued before
            # the first collective feed: the collectives' data movement
            # preempts the dynamic DMA queues, so anything still queued when
            # a collective fires arrives ~30us late.
            for e in range(EC):
                q = nc.sync if e % 2 == 0 else nc.scalar
                q.dma_start(wk[e][:], wkT[e * 128:(e + 1) * 128, :])
                q.dma_start(xk[e][:], xkT[e * 128:(e + 1) * 128, :])
            for e in range(EC):
                nc.sync.dma_start(wv[e][:], wvT[e * 128:(e + 1) * 128, :])
            for e in range(EC):
                nc.sync.dma_start(wq[e][:], wqT[e * 128:(e + 1) * 128, :])
                nc.sync.dma_start(xq[e][:], xqT[e * 128:(e + 1) * 128, :])

            # ---- K^T own half -> ksrc -> AllGather ----
            # e-outer with all 8 PSUM banks live: each arriving (wk,xk)
            # chunk feeds 8 matmuls immediately, so K finishes with the last
            # chunk's arrival instead of trailing the full 4MB load
            with tc.tile_pool(name="psk", bufs=1, space="PSUM") as ppk:
                for wave in range(2):          # d-halves
                    ps = {}
                    for g in range(2):
                        for dd in range(4):
                            ps[(g, dd)] = ppk.tile(
                                [128, 512], F32, name=f"pk{wave}_{g}_{dd}",
                                tag=f"pk{g}_{dd}", bufs=1)
                    for e in range(EC):
                        for g in range(2):
                            for dd in range(4):
                                d = wave * 4 + dd
                                nc.tensor.matmul(
                                    ps[(g, dd)][:],
                                    wk[e][:, d * 128:(d + 1) * 128],
                                    xk[e][:, g * 512:(g + 1) * 512],
                                    start=(e == 0), stop=(e == EC - 1))
                    for g in range(2):
                        for dd in range(4):
                            d = wave * 4 + dd
                            kst = stg.tile([128, 512], BF16, name="kst",
                                           tag="kst", bufs=12)
                            nc.vector.tensor_copy(kst[:], ps[(g, dd)][:])
                            nc.sync.dma_start(
                                ksrc[d * 128:(d + 1) * 128,
                                     g * 512:(g + 1) * 512], kst[:])
            nc.gpsimd.collective_compute(
                "AllGather", mybir.AluOpType.bypass, replica_groups=GROUPS,
                ins=[ksrc[:]], outs=[kdst[:]])

            with tc.tile_pool(name="ps1", bufs=4, space="PSUM") as pp:
                # ---- V own half -> vsrc -> AllGather ----
                for t in range(HS // 128):
                    for h in range(2):
                        ps = pp.tile([128, 512], F32, name=f"pv{t}_{h}", tag="pp")
                        for e in range(EC):
                            nc.tensor.matmul(ps[:], xk[e][:, t * 128:(t + 1) * 128],
                                             wv[e][:, h * 512:(h + 1) * 512],
                                             start=(e == 0), stop=(e == EC - 1))
                        vst = stg.tile([128, 512], BF16, name="vst", tag="vst",
                                       bufs=8)
                        nc.vector.tensor_copy(vst[:], ps[:])
                        nc.sync.dma_start(vsrc[t * 128:(t + 1) * 128,
                                               h * 512:(h + 1) * 512], vst[:])
                nc.gpsimd.collective_compute(
                    "AllGather", mybir.AluOpType.bypass, replica_groups=GROUPS,
                    ins=[vsrc[:]], outs=[vdst[:]])
                # readbacks after both collective feeds are queued (each waits
                # on its collective, must not head-block the vsrc writes)
                for d in range(DC):
                    nc.sync.dma_start(kts[d][:, :HS],
                                      kdst[d * 128:(d + 1) * 128, :])
                    nc.sync.dma_start(kts[d][:, HS:],
                                      kdst[D + d * 128:D + (d + 1) * 128, :])
                for t in range(S // 128):
                    nc.sync.dma_start(vts[t][:], vdst[t * 128:(t + 1) * 128, :])

                # ---- Q^T for own queries, SBUF-resident ----
                for g in range(NQ // 512):
                    for d in range(DC):
                        ps = pp.tile([128, 512], F32, name=f"pq{g}_{d}", tag="pp")
                        for e in range(EC):
                            nc.tensor.matmul(ps[:],
                                             wq[e][:, d * 128:(d + 1) * 128],
                                             xq[e][:, g * 512:(g + 1) * 512],
                                             start=(e == 0), stop=(e == EC - 1))
                        nc.scalar.copy(
                            qt[:, d * NQ + g * 512:d * NQ + (g + 1) * 512],
                            ps[:])

        # ---- attention ----
        # pass 1 (scores/softmax/transpose) needs only K^T and Q; pass 2
        # (O = P^T V / l) needs V, which the second collective delivers
        # roughly when pass 1 ends. Big slots first in pass 1 so their long
        # softmax chains overlap later slots' matmuls; same order in pass 2.
        slot_order = [8, 7, 6, 5, 4, 3, 2, 1]
        with tc.tile_pool(name="att", bufs=1) as ap_, \
             tc.tile_pool(name="ps3", bufs=1, space="PSUM") as pp3:
            for k in slot_order:
                kv = 256 * k
                nch = kv // 128
                ngr = (kv + 511) // 512

                s_ps = [pp3.tile([128, 512], F32, name=f"sps{k}_{g}", tag="sps",
                                 bufs=4) for g in range(ngr)]
                for d in range(DC):
                    lhs = qt[:, d * NQ + (k - 1) * 128:d * NQ + k * 128]
                    for g in range(ngr):
                        w = min(512, kv - g * 512)
                        nc.tensor.matmul(s_ps[g][:, :w], lhs,
                                         kts[d][:, g * 512:g * 512 + w],
                                         start=(d == 0), stop=(d == DC - 1))

                # additive causal mask folded into the last 256 columns
                lg = ngr - 1
                lw = kv - lg * 512
                nc.vector.tensor_add(s_ps[lg][:, lw - 256:lw],
                                     s_ps[lg][:, lw - 256:lw], mask_sb[:])

                # no-max softmax: exp straight from PSUM, row sums via accum
                p_sb = ap_.tile([128, 2048], BF16, name=f"p{k}", tag="p", bufs=2)
                lparts = ap_.tile([128, 4], F32, name=f"lp{k}", tag="lp", bufs=2)
                for g in range(ngr):
                    w = min(512, kv - g * 512)
                    nc.scalar.activation(p_sb[:, g * 512:g * 512 + w],
                                         s_ps[g][:, :w], AF.Exp,
                                         scale=SCALE,
                                         accum_out=lparts[:, g:g + 1])
                lsum = ap_.tile([128, 1], F32, name=f"ls{k}", tag="ls", bufs=2)
                nc.vector.reduce_sum(lsum[:], lparts[:, :ngr], axis=AX.X)
                nc.vector.reciprocal(linvs[k][:], lsum[:])

                for c in range(nch):
                    tps = pp3.tile([128, 128], BF16, name=f"tp{k}_{c}", tag="tps",
                                   bufs=2)
                    nc.tensor.transpose(tps[:], p_sb[:, c * 128:(c + 1) * 128],
                                        ident[:])
                    nc.vector.tensor_copy(pts[k][:, c * 128:(c + 1) * 128], tps[:])

            for k in slot_order:
                nch = 2 * k
                o_ps = [pp3.tile([128, 512], F32, name=f"op{k}_{h}", tag="ops",
                                 bufs=2) for h in range(2)]
                for c in range(nch):
                    lhs = pts[k][:, c * 128:(c + 1) * 128]
                    for h in range(2):
                        nc.tensor.matmul(o_ps[h][:], lhs,
                                         vts[c][:, h * 512:(h + 1) * 512],
                                         start=(c == 0), stop=(c == nch - 1))

                o_sb = ap_.tile([128, D], F32, name=f"o{k}", tag="o", bufs=2)
                for h in range(2):
                    nc.vector.tensor_scalar_mul(o_sb[:, h * 512:(h + 1) * 512],
                                                o_ps[h][:], linvs[k][:])
                nc.sync.dma_start(out[(k - 1) * 128:k * 128, :], o_sb[:])
    if split:
        _split_multi_waits(nc)
    return nc


def _masks():
    j = np.arange(256)[None, :]
    i = np.arange(128)[:, None]
    mask0 = np.where(j <= i, 0.0, MASKVAL).astype(np.float32)
    mask1 = np.where(j <= 128 + i, 0.0, MASKVAL).astype(np.float32)
    return mask0, mask1


def _in_maps(x, w_q, w_k, w_v):
    bf = ml_dtypes.bfloat16
    x = np.asarray(x, dtype=np.float32)
    wqT = np.ascontiguousarray(np.asarray(w_q, np.float32).T).astype(bf)
    wkT = np.ascontiguousarray(np.asarray(w_k, np.float32).T).astype(bf)
    wvT = np.ascontiguousarray(np.asarray(w_v, np.float32).T).astype(bf)
    mask0, mask1 = _masks()

    in_maps = []
    for c in range(NCORES):
        b, p = divmod(c, 2)
        xb = x[b]                                    # [S, E]
        xkT = np.ascontiguousarray(xb[p * HS:(p + 1) * HS, :].T).astype(bf)
        qrows = np.concatenate(
            [xb[128 * (2 * (k - 1) + p):128 * (2 * (k - 1) + p) + 128, :]
             for k in range(1, NSLOT + 1)], axis=0)  # [NQ, E]
        xqT = np.ascontiguousarray(qrows.T).astype(bf)
        in_maps.append({
            "xkT": xkT, "xqT": xqT,
            "wqT": wqT, "wkT": wkT, "wvT": wvT,
            "mask": mask0 if p == 0 else mask1,
        })
    return in_maps


def _scatter(per_core_out):
    out = np.empty((B, S, D), dtype=np.float32)
    for c in range(NCORES):
        b, p = divmod(c, 2)
        oc = per_core_out[c]                         # [NQ, D]
        for k in range(1, NSLOT + 1):
            g = 2 * (k - 1) + p
            out[b, 128 * g:128 * (g + 1), :] = oc[128 * (k - 1):128 * k, :]
    return out


def kernel(x, w_q, w_k, w_v):
    global _prog
    if _prog is None:
        _prog = _build()
    in_maps = _in_maps(x, w_q, w_k, w_v)
    res = run_bass_kernel_spmd(_prog, in_maps, list(range(NCORES)))
    return _scatter([res.results[c]["out"] for c in range(NCORES)])
